# revision 13
# baseline (speedup 1.0000x reference)
"""Trainium2 Bass kernel for ConcentrationLoss.

Math (per batch element b, fully independent across b):
    g      = grid[b] viewed as (2, 4096)            # channels x pixels
    coord1 = g @ aff[b]                             # (2, 4096), the heavy op
    view coord1 as (2, 64, 64); extract 8x8 windows stride 4 -> 15x15 windows
    loss contribution = sum over windows w of [ sum_{p in w} x_p^2 - (sum_{p in w} x_p)^2 / 64 ]
    final = sum_b contribution_b / (8 * 2 * 225 * 64)

Sharding: batch b -> core b (8 cores). Each core streams its 64MB aff slice
through the TensorEngine (memory-bound), reduces the windowed variance on
device to per-channel partial sums, and the host combines the 8 partial
results into the scalar.

Device pipeline per core:
  - Main matmul: out = lhsT.T @ rhs with lhsT = g^T chunks (128, 2) and
    rhs = aff row-band tiles (128, 512), accumulated into PSUM (2, 4096)
    over the 32 contraction chunks. Operands are float32r: single-pass
    matmul at 1 col/cycle (fp32 would stream aff through the PE twice at
    half rate). fp32r rounds operands to ~12 mantissa bits; the end-to-end
    loss error stays ~1e-6 because the truncation noise averages out over
    the 230K-element mean.
  - As soon as PSUM bank n (512 pixels = 8 image rows) finishes
    accumulating, its post-processing overlaps the remaining stream:
    square (ACT), w-direction window sums of x and x^2 (8 strided adds
    each on DVE / GpSimd) into Y/Ysq (2, 64, 15).
  - After the last bank: h-direction window sums (8 strided adds) give
    S/SSq (2, 15, 15); then sum(SSq) and sum(S^2) reduce to a (2, 2)
    output. Host: loss_b = sum_c [ sumSSq_c - sumS2_c / 64 ].
"""

import numpy as np

B = 8
C = 2
H = W = 64
PIX = H * W  # 4096, contraction dim
WIN = 8
STRIDE = 4
OH = OW = 15
KC = PIX // 128  # 32 contraction chunks of 128
NT = PIX // 512  # 8 psum-bank-wide output chunks
ROWS_PER_BANK = 512 // W  # 8 image rows per psum bank
SLAB_BANKS = 2   # psum banks (512-col groups) streamed per column slab
KPT = 4          # contraction chunks per DMA tile within a slab
AFF_BUFS = 6
USE_F32R = True  # fp32r matmul: 1 cycle/col (vs fp32's 4) at ~2.8e-4 operand rounding

_CACHE = {}


def _split_multi_waits(nc, limit=1):
    """The walrus build in this toolchain rejects instructions carrying more
    than one sync wait (any template: CTRL, S3_LW, ...). Tile's scheduler
    freely emits multi-wait instructions. Post-process the scheduled BIR:
    hoist excess waits onto one-wait NoOps inserted immediately before the
    instruction on the same engine (sequencer waits are conjunctive and
    blocking, so semantics are identical)."""
    import concourse.mybir as mybir

    n_split = 0
    for f in nc.m.functions:
        for b in f.blocks:
            insts = b.instructions  # live view
            i = 0
            while i < len(insts):
                inst = insts[i]
                si = inst.sync_info
                if si is not None and len(si.on_wait) > limit:
                    waits = list(si.on_wait)
                    extra, keep = waits[:-limit], waits[-limit:]
                    for w in extra:
                        nop = mybir.InstNoOp(name=f"SWS-{n_split}")
                        n_split += 1
                        nop.engine = inst.engine
                        nop.sync_info = mybir.SyncInfo(on_wait=[w], on_update=[])
                        insts.insert(i, nop)
                        i += 1
                    inst.sync_info = mybir.SyncInfo(
                        on_wait=keep, on_update=si.on_update
                    )
                i += 1
    return n_split


def _build_nc():
    import concourse.bass as bass
    import concourse.mybir as mybir
    import concourse.tile as tile

    f32 = mybir.dt.float32
    fmm = mybir.dt.float32r if USE_F32R else f32
    nc = bass.Bass()
    aff = nc.dram_tensor("aff", [PIX, PIX], fmm, kind="ExternalInput")
    gt = nc.dram_tensor("gt", [128, 2 * KC], fmm, kind="ExternalInput")
    out = nc.dram_tensor("out", [C, 2], f32, kind="ExternalOutput")

    with tile.TileContext(nc) as tc:
        with (
            tc.tile_pool(name="consts", bufs=1) as consts,
            tc.tile_pool(name="small", bufs=1) as small,
            tc.tile_pool(name="sqp", bufs=2) as sqp,
            tc.tile_pool(name="affp", bufs=AFF_BUFS) as affp,
            tc.tile_pool(name="ps1", bufs=1, space="PSUM") as ps1,
        ):
            # consts go through SWDGE (gpsimd) so they never queue behind the
            # big aff stream on the HWDGE rings
            gt_sb = consts.tile([128, 2 * KC], fmm)
            nc.gpsimd.dma_start(out=gt_sb, in_=gt[:, :])

            y_sb = small.tile([C, H, OW], f32)      # w-windowsums of x
            ysq_sb = small.tile([C, H, OW], f32)    # w-windowsums of x^2
            s_sb = small.tile([C, OH * OW], f32)    # full window sums
            ssq_sb = small.tile([C, OH * OW], f32)  # full window sums of x^2
            s2_sb = small.tile([C, OH * OW], f32)   # S^2
            out_sb = small.tile([C, 2], f32)

            c1_ps = ps1.tile([C, PIX], f32)
            ntiles = KC // KPT
            slab_w = SLAB_BANKS * 512

            def bank_postprocess(n):
                """w-direction window sums for psum bank n; overlaps stream."""
                bank = c1_ps[:, n * 512:(n + 1) * 512]
                sq = sqp.tile([C, 512], f32, tag="sq")
                nc.scalar.square(out=sq, in_=bank)
                x4 = bank.rearrange("c (r q s) -> c r q s", r=ROWS_PER_BANK, s=STRIDE)
                q4 = sq.rearrange("c (r q s) -> c r q s", r=ROWS_PER_BANK, s=STRIDE)
                yd = y_sb[:, n * ROWS_PER_BANK:(n + 1) * ROWS_PER_BANK, :]
                qd = ysq_sb[:, n * ROWS_PER_BANK:(n + 1) * ROWS_PER_BANK, :]
                for dw in range(WIN):
                    a, bb = divmod(dw, STRIDE)
                    xs = x4[:, :, a:a + OW, bb]
                    qs = q4[:, :, a:a + OW, bb]
                    if dw == 0:
                        nc.vector.tensor_copy(out=yd, in_=xs)
                        nc.vector.tensor_copy(out=qd, in_=qs)
                    else:
                        nc.vector.tensor_add(out=yd, in0=yd, in1=xs)
                        nc.vector.tensor_add(out=qd, in0=qd, in1=qs)

            # column-slab-major stream: all 32 contraction chunks for one
            # SLAB_BANKS-wide column group, then the next. Banks finish
            # progressively, so their windowed reduction overlaps the stream.
            for s in range(NT // SLAB_BANKS):
                for t in range(ntiles):
                    at = affp.tile([128, KPT, slab_w], fmm)
                    src = aff[
                        t * KPT * 128:(t + 1) * KPT * 128,
                        s * slab_w:(s + 1) * slab_w,
                    ].rearrange("(j p) n -> p j n", p=128)
                    nc.sync.dma_start(out=at, in_=src)
                    for j in range(KPT):
                        kc = t * KPT + j
                        for b in range(SLAB_BANKS):
                            n = SLAB_BANKS * s + b
                            nc.tensor.matmul(
                                c1_ps[:, n * 512:(n + 1) * 512],
                                lhsT=gt_sb[:, 2 * kc:2 * kc + 2],
                                rhs=at[:, j, b * 512:(b + 1) * 512],
                                start=(kc == 0),
                                stop=(kc == KC - 1),
                            )
                            if kc == KC - 1:
                                bank_postprocess(n)

            # h-direction window sums: S[c, i, j] = sum_dh Y[c, 4i+dh, j]
            y4 = y_sb.rearrange("c (q r) j -> c q r j", r=STRIDE)
            q4 = ysq_sb.rearrange("c (q r) j -> c q r j", r=STRIDE)
            sv = s_sb.rearrange("c (i j) -> c i j", j=OW)
            qv = ssq_sb.rearrange("c (i j) -> c i j", j=OW)
            for dh in range(WIN):
                a, bb = divmod(dh, STRIDE)
                ys = y4[:, a:a + OH, bb, :]
                qs = q4[:, a:a + OH, bb, :]
                if dh == 0:
                    nc.vector.tensor_copy(out=sv, in_=ys)
                    nc.vector.tensor_copy(out=qv, in_=qs)
                else:
                    nc.vector.tensor_add(out=sv, in0=sv, in1=ys)
                    nc.vector.tensor_add(out=qv, in0=qv, in1=qs)

            nc.scalar.square(out=s2_sb, in_=s_sb)
            nc.vector.reduce_sum(out=out_sb[:, 0:1], in_=ssq_sb, axis=mybir.AxisListType.X)
            nc.vector.reduce_sum(out=out_sb[:, 1:2], in_=s2_sb, axis=mybir.AxisListType.X)
            nc.sync.dma_start(out=out[:, :], in_=out_sb)
    _split_multi_waits(nc)
    return nc


def _gt_host(grid_b):
    # grid_b: (64, 64, 2). g[c, p] = grid_b.reshape(4096, 2)[p, c]
    # gt layout: gt[p, 2*kc + c] = g[c, 128*kc + p]
    gt = np.ascontiguousarray(grid_b, dtype=np.float32).reshape(PIX, C)
    return np.ascontiguousarray(
        gt.reshape(KC, 128, C).transpose(1, 0, 2).reshape(128, 2 * KC)
    )


def run_cores(aff, grid, trace=False):
    """Compile (cached) and run the per-core bass kernel on cores 0..7.

    Returns the BassKernelResults from run_bass_kernel_spmd."""
    from concourse.bass_utils import run_bass_kernel_spmd

    if "nc" not in _CACHE:
        _CACHE["nc"] = _build_nc()
    nc = _CACHE["nc"]

    in_maps = []
    for b in range(B):
        in_maps.append(
            {
                "aff": np.ascontiguousarray(aff[b], dtype=np.float32),
                "gt": _gt_host(grid[b]),
            }
        )
    return run_bass_kernel_spmd(nc, in_maps, core_ids=list(range(B)), trace=trace)


def kernel(aff, grid):
    aff = np.asarray(aff, dtype=np.float32)
    grid = np.asarray(grid, dtype=np.float32)
    res = run_cores(aff, grid)
    total = 0.0
    for b in range(B):
        o = res.results[b]["out"].astype(np.float64)
        total += o[:, 0].sum() - o[:, 1].sum() / (WIN * WIN)
    total /= B * C * OH * OW * WIN * WIN
    return np.asarray(total, dtype=np.float32)


# revision 14
# speedup vs baseline: 1.1915x; 1.1915x over previous
"""Trainium2 Bass kernel for ConcentrationLoss.

Math (per batch element b, fully independent across b):
    g      = grid[b] viewed as (2, 4096)            # channels x pixels
    coord1 = g @ aff[b]                             # (2, 4096), the heavy op
    view coord1 as (2, 64, 64); extract 8x8 windows stride 4 -> 15x15 windows
    loss contribution = sum over windows w of [ sum_{p in w} x_p^2 - (sum_{p in w} x_p)^2 / 64 ]
    final = sum_b contribution_b / (8 * 2 * 225 * 64)

Sharding: batch b -> core b (8 cores). Each core streams its 64MB aff slice
through the TensorEngine (memory-bound), reduces the windowed variance on
device to per-channel partial sums, and the host combines the 8 partial
results into the scalar.

Device pipeline per core:
  - Main matmul: out = lhsT.T @ rhs with lhsT = g^T chunks (128, 2) and
    rhs = aff row-band tiles (128, 512), accumulated into PSUM (2, 4096)
    over the 32 contraction chunks. Operands are float32r: single-pass
    matmul at 1 col/cycle (fp32 would stream aff through the PE twice at
    half rate). fp32r rounds operands to ~12 mantissa bits; the end-to-end
    loss error stays ~1e-6 because the truncation noise averages out over
    the 230K-element mean.
  - As soon as PSUM bank n (512 pixels = 8 image rows) finishes
    accumulating, its post-processing overlaps the remaining stream:
    square (ACT), w-direction window sums of x and x^2 (8 strided adds
    each on DVE / GpSimd) into Y/Ysq (2, 64, 15).
  - After the last bank: h-direction window sums (8 strided adds) give
    S/SSq (2, 15, 15); then sum(SSq) and sum(S^2) reduce to a (2, 2)
    output. Host: loss_b = sum_c [ sumSSq_c - sumS2_c / 64 ].
"""

import numpy as np

B = 8
C = 2
H = W = 64
PIX = H * W  # 4096, contraction dim
WIN = 8
STRIDE = 4
OH = OW = 15
KC = PIX // 128  # 32 contraction chunks of 128
NT = PIX // 512  # 8 psum-bank-wide output chunks
ROWS_PER_BANK = 512 // W  # 8 image rows per psum bank
SLAB_BANKS = 4   # psum banks (512-col groups) streamed per column slab
KPT = 2          # contraction chunks per DMA tile within a slab
AFF_BUFS = 6
USE_F32R = True  # fp32r matmul: 1 cycle/col (vs fp32's 4) at ~2.8e-4 operand rounding

_CACHE = {}


def _split_multi_waits(nc, limit=1):
    """The walrus build in this toolchain rejects instructions carrying more
    than one sync wait (any template: CTRL, S3_LW, ...). Tile's scheduler
    freely emits multi-wait instructions. Post-process the scheduled BIR:
    hoist excess waits onto one-wait NoOps inserted immediately before the
    instruction on the same engine (sequencer waits are conjunctive and
    blocking, so semantics are identical)."""
    import concourse.mybir as mybir

    n_split = 0
    for f in nc.m.functions:
        for b in f.blocks:
            insts = b.instructions  # live view
            i = 0
            while i < len(insts):
                inst = insts[i]
                si = inst.sync_info
                if si is not None and len(si.on_wait) > limit:
                    waits = list(si.on_wait)
                    extra, keep = waits[:-limit], waits[-limit:]
                    for w in extra:
                        nop = mybir.InstNoOp(name=f"SWS-{n_split}")
                        n_split += 1
                        nop.engine = inst.engine
                        nop.sync_info = mybir.SyncInfo(on_wait=[w], on_update=[])
                        insts.insert(i, nop)
                        i += 1
                    inst.sync_info = mybir.SyncInfo(
                        on_wait=keep, on_update=si.on_update
                    )
                i += 1
    return n_split


def _build_nc():
    import concourse.bass as bass
    import concourse.mybir as mybir
    import concourse.tile as tile

    f32 = mybir.dt.float32
    fmm = mybir.dt.float32r if USE_F32R else f32
    nc = bass.Bass()
    aff = nc.dram_tensor("aff", [PIX, PIX], fmm, kind="ExternalInput")
    gt = nc.dram_tensor("gt", [128, 2 * KC], fmm, kind="ExternalInput")
    out = nc.dram_tensor("out", [C, 2], f32, kind="ExternalOutput")

    with tile.TileContext(nc) as tc:
        with (
            tc.tile_pool(name="consts", bufs=1) as consts,
            tc.tile_pool(name="small", bufs=1) as small,
            tc.tile_pool(name="sqp", bufs=2) as sqp,
            tc.tile_pool(name="affp", bufs=AFF_BUFS) as affp,
            tc.tile_pool(name="ps1", bufs=1, space="PSUM") as ps1,
        ):
            # consts go through SWDGE (gpsimd) so they never queue behind the
            # big aff stream on the HWDGE rings
            gt_sb = consts.tile([128, 2 * KC], fmm)
            nc.gpsimd.dma_start(out=gt_sb, in_=gt[:, :])

            y_sb = small.tile([C, H, OW], f32)      # w-windowsums of x
            ysq_sb = small.tile([C, H, OW], f32)    # w-windowsums of x^2
            s_sb = small.tile([C, OH * OW], f32)    # full window sums
            ssq_sb = small.tile([C, OH * OW], f32)  # full window sums of x^2
            s2_sb = small.tile([C, OH * OW], f32)   # S^2
            out_sb = small.tile([C, 2], f32)

            c1_ps = ps1.tile([C, PIX], f32)
            ntiles = KC // KPT
            slab_w = SLAB_BANKS * 512

            def windowed(ap, row_step, n_rows):
                """4-dim overlapping AP: [part, row, window j, dw] over a
                (C, n_rows*row_step) region; one tensor_reduce(X) gives the
                w-direction window sums in a single instruction."""
                return bass.AP(
                    tensor=ap.tensor,
                    offset=ap.offset,
                    ap=[list(ap.ap[0]), [row_step, n_rows], [STRIDE, OW], [1, WIN]],
                )

            def bank_postprocess(n):
                """w-direction window sums for psum bank n; overlaps stream."""
                bank = c1_ps[:, n * 512:(n + 1) * 512]
                sq = sqp.tile([C, 512], f32, tag="sq")
                nc.scalar.square(out=sq, in_=bank)
                yd = y_sb[:, n * ROWS_PER_BANK:(n + 1) * ROWS_PER_BANK, :]
                qd = ysq_sb[:, n * ROWS_PER_BANK:(n + 1) * ROWS_PER_BANK, :]
                nc.vector.reduce_sum(
                    out=yd, in_=windowed(bank, W, ROWS_PER_BANK),
                    axis=mybir.AxisListType.X,
                )
                nc.vector.reduce_sum(
                    out=qd, in_=windowed(sq[:, :], W, ROWS_PER_BANK),
                    axis=mybir.AxisListType.X,
                )

            # column-slab-major stream: all 32 contraction chunks for one
            # SLAB_BANKS-wide column group, then the next. Banks finish
            # progressively, so their windowed reduction overlaps the stream.
            for s in range(NT // SLAB_BANKS):
                for t in range(ntiles):
                    at = affp.tile([128, KPT, slab_w], fmm)
                    src = aff[
                        t * KPT * 128:(t + 1) * KPT * 128,
                        s * slab_w:(s + 1) * slab_w,
                    ].rearrange("(j p) n -> p j n", p=128)
                    nc.sync.dma_start(out=at, in_=src)
                    for j in range(KPT):
                        kc = t * KPT + j
                        for b in range(SLAB_BANKS):
                            n = SLAB_BANKS * s + b
                            nc.tensor.matmul(
                                c1_ps[:, n * 512:(n + 1) * 512],
                                lhsT=gt_sb[:, 2 * kc:2 * kc + 2],
                                rhs=at[:, j, b * 512:(b + 1) * 512],
                                start=(kc == 0),
                                stop=(kc == KC - 1),
                            )
                            if kc == KC - 1:
                                bank_postprocess(n)

            # h-direction window sums: S[c, i, j] = sum_dh Y[c, 4i+dh, j]
            # single windowed reduce per quantity: AP dims (i, j, dh)
            def h_windowed(ap):
                return bass.AP(
                    tensor=ap.tensor,
                    offset=ap.offset,
                    ap=[list(ap.ap[0]), [STRIDE * OW, OH], [1, OW], [OW, WIN]],
                )

            sv = s_sb.rearrange("c (i j) -> c i j", j=OW)
            qv = ssq_sb.rearrange("c (i j) -> c i j", j=OW)
            nc.vector.reduce_sum(
                out=sv, in_=h_windowed(y_sb[:, :, :]), axis=mybir.AxisListType.X
            )
            nc.vector.reduce_sum(
                out=qv, in_=h_windowed(ysq_sb[:, :, :]), axis=mybir.AxisListType.X
            )

            nc.scalar.square(out=s2_sb, in_=s_sb)
            nc.vector.reduce_sum(out=out_sb[:, 0:1], in_=ssq_sb, axis=mybir.AxisListType.X)
            nc.vector.reduce_sum(out=out_sb[:, 1:2], in_=s2_sb, axis=mybir.AxisListType.X)
            nc.sync.dma_start(out=out[:, :], in_=out_sb)
    _split_multi_waits(nc)
    return nc


def _gt_host(grid_b):
    # grid_b: (64, 64, 2). g[c, p] = grid_b.reshape(4096, 2)[p, c]
    # gt layout: gt[p, 2*kc + c] = g[c, 128*kc + p]
    gt = np.ascontiguousarray(grid_b, dtype=np.float32).reshape(PIX, C)
    return np.ascontiguousarray(
        gt.reshape(KC, 128, C).transpose(1, 0, 2).reshape(128, 2 * KC)
    )


def run_cores(aff, grid, trace=False):
    """Compile (cached) and run the per-core bass kernel on cores 0..7.

    Returns the BassKernelResults from run_bass_kernel_spmd."""
    from concourse.bass_utils import run_bass_kernel_spmd

    if "nc" not in _CACHE:
        _CACHE["nc"] = _build_nc()
    nc = _CACHE["nc"]

    in_maps = []
    for b in range(B):
        in_maps.append(
            {
                "aff": np.ascontiguousarray(aff[b], dtype=np.float32),
                "gt": _gt_host(grid[b]),
            }
        )
    return run_bass_kernel_spmd(nc, in_maps, core_ids=list(range(B)), trace=trace)


def kernel(aff, grid):
    aff = np.asarray(aff, dtype=np.float32)
    grid = np.asarray(grid, dtype=np.float32)
    res = run_cores(aff, grid)
    total = 0.0
    for b in range(B):
        o = res.results[b]["out"].astype(np.float64)
        total += o[:, 0].sum() - o[:, 1].sum() / (WIN * WIN)
    total /= B * C * OH * OW * WIN * WIN
    return np.asarray(total, dtype=np.float32)
